# revision 36
# baseline (speedup 1.0000x reference)
"""Trainium2 Bass kernel for nn_MultiHeadAttention_377957122345.

B=16, T=512, C=1024, H=16, D=64.  Data-parallel over batch: each of the
8 NeuronCores computes attention for 2 sequences; no collectives.

Per-core device program (SPMD, identical on all cores):
  - inputs staged on host pre-cast to bf16 in transposed layouts: xT
    [C, NT] (c_in on partitions), W^T [c_in, c_out] for all four
    projections (Wq/bq pre-scaled by 1/sqrt(D)); rel_pos_bias is
    exp()'d with the causal mask folded in multiplicatively (0 above
    the diagonal) and packed per head pair as one contiguous
    [128, 2560] panel (lower-tri blocks at offsets 0/512/896/1152 per
    parity) so each pair costs one DMA and one elementwise multiply.
  - all matmuls bf16 with fp32 PSUM accumulation; exp on ScalarE.
  - Q/K projections produce q^T/k^T (head_dim on partitions); V is
    produced ones-augmented (head h in columns [65h, 65h+64), ones at
    65h+64) so the AV matmul emits the softmax denominator as row 64.
  - S computed transposed ([s on partitions, t]) with causal block
    skipping; the two heads of a pair live on partition halves 0:64 /
    64:128 and their S matmuls are issued adjacently so the PE can run
    them concurrently in disjoint row-groups.
  - AV per head: 4 matmuls accumulate into ONE [65, 512] PSUM tile via
    column-offset accumulation (key block j covers queries t >= 128j);
    row 64 accumulates the denominator l.
  - softmax normalization: both heads' 1/l rows (DVE reciprocal, bf16)
    pack into one [1, 1024] tile, round-trip through a DRAM scratch,
    and return as two partition-broadcast DMA reads (step-0 partition
    dim is only legal from DRAM); one aligned DVE multiply per head
    then writes normalized o^T.
  - emission order interleaves phases so ScalarE exp work (~85us) hides
    under projection PE work: V(seq0); per pair m {Q(m), K(m), one
    V(seq1) tile, attention(m, b=0)}; per pair m {attention(m, b=1),
    one seq-0 out-proj tile}; seq-1 out-proj.  Input loads are split
    across both HWDGE rings (SP + ACT).
  - y is written bf16 (host casts back to fp32; tolerance is 2e-2).

Optimization passes layered on the above (this session):
  - projection/out-proj emitters run ko-outer / chunk-inner so consecutive
    matmuls share each stationary; a post-build pass (_dedup_ldweights)
    removes the redundant InstLdweights (the PE keeps the loaded
    stationary across matmuls -- verified on HW with divergent weights),
    merging their sync info into the matmul.  ~220-250 of 896 weight
    loads disappear.
  - inputs are loaded as one tile per 128-row block (xT/wq/wk/wv/wo) so
    the first matmuls unblock after the first DMA, not the eighth; rings
    rebalanced (sync: xT+wk+biasm+y, scalar: bv+wv+wq+wo+bo) to match
    first-use times.
  - oT is split per sequence so iteration i+1's b=0 attention writes
    don't serialize against iteration i's seq-1 out-projection reads in
    the steady-state loop.
  - optional wide (2-PSUM-bank, [*, 1024]) tiles per stage
    (wide_proj/wide_s/wide_av) merge the two 512-col drains into one
    ACT/DVE instruction (exp count halved, one reciprocal per pair);
    costs PSUM flexibility -- flag choice decided empirically on HW.
"""

import numpy as np

B, T, C, H = 16, 512, 1024, 16
D = C // H  # 64
N_CORES = 8
B_LOC = B // N_CORES  # 2 sequences per core
NT = B_LOC * T  # 1024 tokens per core
P = 128
KO = C // P  # 8 contraction subtiles
TB = T // P  # 4 query blocks per sequence
NEG = -1e30
SPLIT_LIMIT = 1

_CACHE = {}

import contextlib


def _nullcm():
    return contextlib.nullcontext()



def _split_big_waits(nc, mybir, limit=1):
    # This walrus build rejects instructions whose sync_info.on_wait
    # exceeds its slot count (the Tile end-of-kernel Drain trips it).
    # Move excess waits onto dedicated same-engine NOPs placed directly
    # before the instruction; the engine stalls on those first, so the
    # semantics are unchanged.
    f = nc.m.functions[0]
    for bb in f.blocks:
        out = []
        changed = False
        for inst in bb.instructions:
            si = getattr(inst, "sync_info", None)
            waits = list(si.on_wait) if si is not None and si.on_wait else []
            if len(waits) > limit:
                changed = True
                head, tail = waits[:-limit], waits[-limit:]
                for k in range(0, len(head), limit):
                    out.append(
                        mybir.InstNoOp(
                            name=f"I-{nc.next_id()}",
                            sync_info=mybir.SyncInfo(
                                on_wait=head[k : k + limit], on_update=[]
                            ),
                            bass_nofuse=True,
                            engine=inst.engine,
                        )
                    )
                si.on_wait = tail
            out.append(inst)
        if changed:
            bb.instructions = out
    return nc


def _dedup_ldweights(nc, mybir, groups):
    """Remove each InstLdweights whose stationary access pattern equals the
    one already loaded into the PE array, merging its sync info into its
    matmul.  Valid because the PE keeps the loaded stationary across
    matmuls (verified on HW).  `groups` maps matmul name -> emitter-call id;
    dedup only applies when the previous matmul came from the same emitter
    call, which guarantees no intervening write to the stationary's SBUF
    region (all writes to an emitter's stationaries precede its reads, and
    groups never span loop iterations)."""
    found = 0
    f = nc.m.functions[0]
    for bb in f.blocks:
        pending = None  # most recent InstLdweights not yet consumed
        loaded_ap = None  # AP string of the weights currently in the array
        last_group = None  # emitter group of the matmul that loaded them
        drop = set()
        merge = {}  # matmul name -> ldweights inst to merge
        for inst in bb.instructions:
            tn = type(inst).__name__
            if tn == "InstLdweights":
                pending = inst
            elif tn == "InstMatmult":
                g = groups.get(inst.name)
                if (
                    pending is not None
                    and loaded_ap is not None
                    and g is not None
                    and g == last_group
                    and str(pending.ins[0]) == loaded_ap
                ):
                    drop.add(id(pending))
                    merge[inst.name] = pending
                    found += 1
                elif pending is not None:
                    loaded_ap = str(pending.ins[0])
                last_group = g
                pending = None
        if not drop:
            continue
        out = []
        for inst in bb.instructions:
            if id(inst) in drop:
                continue
            if type(inst).__name__ == "InstMatmult" and inst.name in merge:
                ld = merge[inst.name]
                lsi = ld.sync_info
                si = inst.sync_info
                if lsi is not None and (lsi.on_wait or lsi.on_update):
                    if si is None:
                        inst.sync_info = mybir.SyncInfo(
                            on_wait=list(lsi.on_wait), on_update=list(lsi.on_update)
                        )
                    else:
                        si.on_wait = list(lsi.on_wait) + list(si.on_wait)
                        si.on_update = list(si.on_update) + list(lsi.on_update)
            out.append(inst)
        bb.instructions = out
    return found


def build_program(split_waits=True, reps=1, skip_attn=False, skip_proj=False,
                  skip_oproj=False, skip_loads=False, attn_lvl=4,
                  dedup_ldw=True, holdback=1, pair_proj=True,
                  psp_bufs=2, pss_bufs=2, pso_bufs=2, share_s=True,
                  wide_proj=True, wide_s=True, wide_av=True):
    import concourse.bass as bass
    import concourse.mybir as mybir
    import concourse.tile as tile

    fp32 = mybir.dt.float32
    bf16 = mybir.dt.bfloat16
    Act = mybir.ActivationFunctionType

    NCH = NT // 512  # 2 free-dim chunks of 512

    nc = bass.Bass()
    xT = nc.dram_tensor("xT", [C, NT], bf16, kind="ExternalInput")
    wqT = nc.dram_tensor("wqT", [C, C], bf16, kind="ExternalInput")
    wkT = nc.dram_tensor("wkT", [C, C], bf16, kind="ExternalInput")
    wvT = nc.dram_tensor("wvT", [C, C], bf16, kind="ExternalInput")
    woT = nc.dram_tensor("woT", [C, C], bf16, kind="ExternalInput")
    bqd = nc.dram_tensor("bq", [C], fp32, kind="ExternalInput")
    bkd = nc.dram_tensor("bk", [C], fp32, kind="ExternalInput")
    bvd = nc.dram_tensor("bv", [C], fp32, kind="ExternalInput")
    bod = nc.dram_tensor("bo", [C], fp32, kind="ExternalInput")
    biasm = nc.dram_tensor("biasm", [H // 2, P, 2560], bf16, kind="ExternalInput")
    r_dram = nc.dram_tensor("r_scr", [H // 2, B_LOC, 1024], bf16)
    y = nc.dram_tensor("y", [NT, C], bf16, kind="ExternalOutput")

    # matmul name -> emitter-call id, for the Ldweights dedup pass
    dedup_groups = {}
    group_ctr = [0]

    with tile.TileContext(nc) as tc, \
         tc.tile_pool(name="consts", bufs=1) as consts, \
         tc.tile_pool(name="persist", bufs=1) as persist, \
         tc.tile_pool(name="biasp", bufs=1) as biasp, \
         tc.tile_pool(name="soft", bufs=1) as soft, \
         tc.tile_pool(name="ypool", bufs=3) as ypool, \
         tc.tile_pool(name="psP", bufs=psp_bufs, space="PSUM") as psP, \
         (contextlib.nullcontext(psP) if share_s else
          tc.tile_pool(name="psS", bufs=pss_bufs, space="PSUM")) as psSp, \
         tc.tile_pool(name="psO", bufs=1, space="PSUM") as psO, \
         (tc.For_i(0, reps, 1) if reps > 1 else _nullcm()):

        # ----- constants -----
        bq_sb = consts.tile([P, KO], fp32, name="bq_sb")
        nc.sync.dma_start(out=bq_sb, in_=bqd.rearrange("(o p) -> p o", p=P))
        bk_sb = consts.tile([P, KO], fp32, name="bk_sb")
        nc.sync.dma_start(out=bk_sb, in_=bkd.rearrange("(o p) -> p o", p=P))
        bv_sb = consts.tile([P, C], fp32, name="bv_sb")
        bo_sb = consts.tile([P, C], fp32, name="bo_sb")
        # ----- load inputs (already bf16 on host), one tile per ko so
        # consumers unblock as each 128-row block lands -----
        def load(dram, name, eng):
            tiles = []
            for ko in range(KO):
                t = persist.tile([P, C], bf16, name=f"{name}{ko}")
                if not skip_loads or ko == 0:
                    eng.dma_start(out=t, in_=dram[ko * P : (ko + 1) * P, :])
                tiles.append(t)
            return tiles

        bv_ap = bvd[:]
        nc.scalar.dma_start(
            out=bv_sb,
            in_=bass.AP(tensor=bv_ap.tensor, offset=bv_ap.offset, ap=[[0, P], [1, C]]),
        )
        # ring split tuned for first-use times: sync carries xT+wk (then the
        # per-pair biasm panels and y stores); scalar carries wv+wq+wo
        xT_bf = load(xT, "xT_bf", nc.sync)
        wv_bf = load(wvT, "wv_bf", nc.scalar)
        wk_bf = load(wkT, "wk_bf", nc.sync)
        wq_bf = load(wqT, "wq_bf", nc.scalar)
        wo_bf = load(woT, "wo_bf", nc.scalar)
        bo_ap = bod[:]
        nc.scalar.dma_start(
            out=bo_sb,
            in_=bass.AP(tensor=bo_ap.tensor, offset=bo_ap.offset, ap=[[0, P], [1, C]]),
        )

        qT_bf = persist.tile([P, KO, NT], bf16, name="qT_bf")
        kT_bf = persist.tile([P, KO, NT], bf16, name="kT_bf")
        DA = D + 1  # 65: head dim + ones column
        vaug = persist.tile([P, NT // P, H * DA], bf16, name="vaug")
        # one oT tile per sequence: next iteration's b=0 attention writes
        # don't contend with this iteration's seq-1 out-projection reads
        oT_bt = [persist.tile([P, KO, T], bf16, name=f"oT_b{b}")
                 for b in range(B_LOC)]

        # ones columns of vaug (only the 16 columns at 65h+64 per block)
        va = vaug[:]
        nc.gpsimd.memset(
            bass.AP(
                tensor=va.tensor,
                offset=va.offset + D,
                ap=[[(NT // P) * H * DA, P], [H * DA, NT // P], [DA, H]],
            ),
            1.0,
        )

        # ----- projection pair emitters: ko-outer / nch-inner so the two
        # chunks' matmuls share each stationary load (Ldweights deduped)
        def qk_tile(w_bf, out_bf, b_sb, mo):
            group_ctr[0] += 1
            g = group_ctr[0]
            if pair_proj:
                if wide_proj:
                    # one 2-bank psum tile; the two 512-col chains land in
                    # adjacent banks, drained by a single wide activation
                    ps = psP.tile([P, 1024], fp32, tag="psP", name="ps_qk")
                    views = [ps[:, nch * 512 : (nch + 1) * 512] for nch in range(NCH)]
                else:
                    pss = [psP.tile([P, 512], fp32, tag="psP", name="ps_qk")
                           for _ in range(NCH)]
                    views = [t[:] for t in pss]
                for ko in range(KO):
                    for nch in range(NCH):
                        mm = nc.tensor.matmul(
                            views[nch],
                            lhsT=w_bf[ko][:, mo * P : (mo + 1) * P],
                            rhs=xT_bf[ko][:, nch * 512 : (nch + 1) * 512],
                            start=(ko == 0),
                            stop=(ko == KO - 1),
                            skip_group_check=wide_proj,
                        )
                        dedup_groups[mm.ins.name] = g
                if wide_proj:
                    nc.scalar.activation(
                        out=out_bf[:, mo, :],
                        in_=ps,
                        func=Act.Identity,
                        bias=b_sb[:, mo : mo + 1],
                    )
                else:
                    for nch in range(NCH):
                        nc.scalar.activation(
                            out=out_bf[:, mo, nch * 512 : (nch + 1) * 512],
                            in_=views[nch],
                            func=Act.Identity,
                            bias=b_sb[:, mo : mo + 1],
                        )
                return
            for nch in range(NCH):
                ps = psP.tile([P, 512], fp32, tag="psP", name="ps_qk")
                for ko in range(KO):
                    mm = nc.tensor.matmul(
                        ps,
                        lhsT=w_bf[ko][:, mo * P : (mo + 1) * P],
                        rhs=xT_bf[ko][:, nch * 512 : (nch + 1) * 512],
                        start=(ko == 0),
                        stop=(ko == KO - 1),
                    )
                    dedup_groups[mm.ins.name] = g
                nc.scalar.activation(
                    out=out_bf[:, mo, nch * 512 : (nch + 1) * 512],
                    in_=ps,
                    func=Act.Identity,
                    bias=b_sb[:, mo : mo + 1],
                )

        def v_tile(to):
            group_ctr[0] += 1
            g = group_ctr[0]
            VROW = (NT // P) * H * DA  # vaug per-partition extent
            if pair_proj:
                if wide_proj:
                    ps = psP.tile([P, 1024], fp32, tag="psP", name="ps_v")
                    views = [ps[:, nch * 512 : (nch + 1) * 512] for nch in range(NCH)]
                else:
                    pss = [psP.tile([P, 512], fp32, tag="psP", name="ps_v")
                           for _ in range(NCH)]
                    views = [t[:] for t in pss]
                for ko in range(KO):
                    for nch in range(NCH):
                        mm = nc.tensor.matmul(
                            views[nch],
                            lhsT=xT_bf[ko][:, to * P : (to + 1) * P],
                            rhs=wv_bf[ko][:, nch * 512 : (nch + 1) * 512],
                            start=(ko == 0),
                            stop=(ko == KO - 1),
                            skip_group_check=wide_proj,
                        )
                        dedup_groups[mm.ins.name] = g
                va2 = vaug[:]
                bv_ap2 = bv_sb[:]
                if wide_proj:
                    # one strided add writes all 16 heads' D-cols of the block
                    ps_ap = ps[:]
                    nc.vector.tensor_add(
                        out=bass.AP(
                            tensor=va2.tensor,
                            offset=va2.offset + to * H * DA,
                            ap=[[VROW, P], [DA, H], [1, D]],
                        ),
                        in0=bass.AP(
                            tensor=ps_ap.tensor,
                            offset=ps_ap.offset,
                            ap=[[1024, P], [D, H], [1, D]],
                        ),
                        in1=bass.AP(
                            tensor=bv_ap2.tensor,
                            offset=bv_ap2.offset,
                            ap=[[C, P], [D, H], [1, D]],
                        ),
                    )
                else:
                    for nch in range(NCH):
                        ps_ap = views[nch]
                        nc.vector.tensor_add(
                            out=bass.AP(
                                tensor=va2.tensor,
                                offset=va2.offset + to * H * DA + nch * 8 * DA,
                                ap=[[VROW, P], [DA, 8], [1, D]],
                            ),
                            in0=bass.AP(
                                tensor=ps_ap.tensor,
                                offset=ps_ap.offset,
                                ap=[[512, P], [D, 8], [1, D]],
                            ),
                            in1=bass.AP(
                                tensor=bv_ap2.tensor,
                                offset=bv_ap2.offset + nch * 512,
                                ap=[[C, P], [D, 8], [1, D]],
                            ),
                        )
                return
            for nch in range(NCH):
                ps = psP.tile([P, 512], fp32, tag="psP", name="ps_v")
                for ko in range(KO):
                    mm = nc.tensor.matmul(
                        ps,
                        lhsT=xT_bf[ko][:, to * P : (to + 1) * P],
                        rhs=wv_bf[ko][:, nch * 512 : (nch + 1) * 512],
                        start=(ko == 0),
                        stop=(ko == KO - 1),
                    )
                    dedup_groups[mm.ins.name] = g
                # one strided add writes all 8 heads' D-columns of this chunk
                va2 = vaug[:]
                ps_ap = ps[:]
                bv_ap2 = bv_sb[:]
                nc.vector.tensor_add(
                    out=bass.AP(
                        tensor=va2.tensor,
                        offset=va2.offset + to * H * DA + nch * 8 * DA,
                        ap=[[VROW, P], [DA, 8], [1, D]],
                    ),
                    in0=bass.AP(
                        tensor=ps_ap.tensor,
                        offset=ps_ap.offset,
                        ap=[[512, P], [D, 8], [1, D]],
                    ),
                    in1=bass.AP(
                        tensor=bv_ap2.tensor,
                        offset=bv_ap2.offset + nch * 512,
                        ap=[[C, P], [D, 8], [1, D]],
                    ),
                )

        if skip_proj:
            nc.vector.memset(qT_bf[:], 0.0)
            nc.vector.memset(kT_bf[:], 0.0)
            nc.vector.memset(vaug[:], 1.0)
        else:
            # V for sequence 0 up front; everything else interleaved below
            for to in range(TB):
                v_tile(to)

        # ----- attention pair emitter (parity = partition half) -----
        if skip_attn:
            attn_lvl = 0
        if attn_lvl < 4:
            for _ob in oT_bt:
                nc.vector.memset(_ob[:], 0.0)

        def attn_pair(m, b):
                ebp = None
                if attn_lvl >= 2:
                    ebp = biasp.tile([P, 2560], bf16, tag="bias", bufs=4, name="ebp")
                    nc.sync.dma_start(out=ebp, in_=biasm[m])
                OFF = [0, 512, 896, 1152]
                PTb = soft.tile([P, 2560], bf16, tag="PT", bufs=3)
                for j in range(TB):
                    wj = T - j * P
                    if wide_s:
                        # both parities' S blocks land in the two banks of one
                        # wide psum tile, drained by a single strided exp
                        psS = psSp.tile([P, 1024], fp32,
                                        tag=("psP" if share_s else "psS"))
                        for parity in range(2):
                            po = parity * D
                            kh = kT_bf[po : po + D, m, b * T + j * P : b * T + (j + 1) * P]
                            qh = qT_bf[po : po + D, m, b * T + j * P : (b + 1) * T]
                            nc.tensor.matmul(
                                psS[:, parity * 512 : parity * 512 + wj],
                                lhsT=kh, rhs=qh, start=True, stop=True,
                                skip_group_check=True,
                            )
                        ps_ap = psS[:]
                        ptb_ap = PTb[:]
                        PTROW = 2560
                        nc.scalar.activation(
                            out=bass.AP(
                                tensor=ptb_ap.tensor,
                                offset=ptb_ap.offset + OFF[j],
                                ap=[[PTROW, P], [1280, 2], [1, wj]],
                            ),
                            in_=bass.AP(
                                tensor=ps_ap.tensor,
                                offset=ps_ap.offset,
                                ap=[[1024, P], [512, 2], [1, wj]],
                            ),
                            func=Act.Exp,
                        )
                        continue
                    for parity in range(2):
                        po = parity * D
                        kh = kT_bf[po : po + D, m, b * T + j * P : b * T + (j + 1) * P]
                        qh = qT_bf[po : po + D, m, b * T + j * P : (b + 1) * T]
                        psS = psSp.tile([P, 512], fp32,
                                        tag=("psP" if share_s else "psS"))
                        nc.tensor.matmul(
                            psS[:, :wj], lhsT=kh, rhs=qh, start=True, stop=True
                        )
                        nc.scalar.activation(
                            out=PTb[:, parity * 1280 + OFF[j] : parity * 1280 + OFF[j] + wj],
                            in_=psS[:, :wj],
                            func=Act.Exp,
                        )
                if attn_lvl >= 2:
                    nc.vector.tensor_mul(out=PTb[:], in0=PTb[:], in1=ebp[:])
                PTs = {(parity, j): PTb[:, parity * 1280 + OFF[j] : parity * 1280 + OFF[j] + (T - j * P)]
                       for parity in range(2) for j in range(TB)}
                if attn_lvl < 3:
                    return
                r01 = soft.tile([1, 1024], bf16, tag="r", bufs=4)
                if wide_av:
                    # both heads' AV accumulate into the two banks of one wide
                    # psum tile; the denominator rows sit side by side so one
                    # reciprocal covers the pair
                    pso = psO.tile([DA, 1024], fp32, tag="psO", bufs=pso_bufs)
                    pviews = [pso[:, parity * 512 : (parity + 1) * 512]
                              for parity in range(2)]
                else:
                    psos = [psO.tile([DA, 512], fp32, tag="psO", bufs=pso_bufs,
                                     name="pso")
                            for _ in range(2)]
                    pviews = [t[:] for t in psos]
                for parity in range(2):
                    h = 2 * m + parity
                    base = parity * 512 if wide_av else 0
                    tgt = pso if wide_av else psos[parity]
                    for j in range(TB):
                        nc.tensor.matmul(
                            tgt[:, base + j * P : base + 512],
                            lhsT=vaug[:, b * TB + j, h * DA : (h + 1) * DA],
                            rhs=PTs[(parity, j)],
                            start=(j == 0),
                            stop=(j == TB - 1),
                            skip_group_check=True,
                        )
                if attn_lvl < 4:
                    for parity in range(2):
                        nc.vector.tensor_copy(
                            out=oT_bt[b][parity * D : parity * D + D, m, :],
                            in_=pviews[parity][:D, :],
                        )
                    return
                rd = r_dram[:]
                with nc.allow_low_precision(reason="1/l broadcast in bf16"):
                    if wide_av:
                        nc.vector.reciprocal(out=r01[:], in_=pso[D : D + 1, :])
                    else:
                        for parity in range(2):
                            nc.vector.reciprocal(
                                out=r01[0:1, parity * 512 : (parity + 1) * 512],
                                in_=pviews[parity][D : D + 1, :],
                            )
                if wide_av:
                    nc.sync.dma_start(
                        out=bass.AP(
                            tensor=rd.tensor,
                            offset=rd.offset + (m * B_LOC + b) * 1024,
                            ap=[[1, 1], [1, 1024]],
                        ),
                        in_=r01[:],
                    )
                else:
                    for parity in range(2):
                        nc.sync.dma_start(
                            out=bass.AP(
                                tensor=rd.tensor,
                                offset=rd.offset + (m * B_LOC + b) * 1024 + parity * 512,
                                ap=[[1, 1], [1, 512]],
                            ),
                            in_=r01[0:1, parity * 512 : (parity + 1) * 512],
                        )
                rbs = {}
                for parity in range(2):
                    rb = soft.tile([D, 512], bf16, tag=f"rb{parity}", bufs=3)
                    nc.sync.dma_start(
                        out=rb,
                        in_=bass.AP(
                            tensor=rd.tensor,
                            offset=rd.offset + (m * B_LOC + b) * 1024 + parity * 512,
                            ap=[[0, D], [1, 512]],
                        ),
                    )
                    rbs[parity] = rb
                for parity in range(2):
                    nc.vector.tensor_mul(
                        out=oT_bt[b][parity * D : parity * D + D, m, :],
                        in0=pviews[parity][:D, :],
                        in1=rbs[parity],
                    )

        # ----- output projection pair emitter (both nch chunks of one
        # token block; stationary oT[co, to] shared between the chunks)
        def oproj_tile(to):
            group_ctr[0] += 1
            g = group_ctr[0]
            if pair_proj:
                if wide_proj:
                    ps = psP.tile([P, 1024], fp32, tag="psP", name="ps_o")
                    views = [ps[:, nch * 512 : (nch + 1) * 512] for nch in range(NCH)]
                else:
                    pss = [psP.tile([P, 512], fp32, tag="psP", name="ps_o")
                           for _ in range(NCH)]
                    views = [t[:] for t in pss]
                for co in range(KO):
                    for nch in range(NCH):
                        mm = nc.tensor.matmul(
                            views[nch],
                            lhsT=oT_bt[to // TB][:, co, (to % TB) * P : (to % TB + 1) * P],
                            rhs=wo_bf[co][:, nch * 512 : (nch + 1) * 512],
                            start=(co == 0),
                            stop=(co == KO - 1),
                            skip_group_check=wide_proj,
                        )
                        dedup_groups[mm.ins.name] = g
                if wide_proj:
                    ysb = ypool.tile([P, 1024], bf16, tag="y", name="ysb")
                    nc.vector.tensor_add(out=ysb, in0=ps, in1=bo_sb)
                    nc.sync.dma_start(
                        out=y[to * P : (to + 1) * P, :],
                        in_=ysb,
                    )
                else:
                    for nch in range(NCH):
                        ysb = ypool.tile([P, 512], bf16, tag="y", name="ysb")
                        nc.vector.tensor_add(
                            out=ysb, in0=views[nch],
                            in1=bo_sb[:, nch * 512 : (nch + 1) * 512],
                        )
                        nc.sync.dma_start(
                            out=y[to * P : (to + 1) * P, nch * 512 : (nch + 1) * 512],
                            in_=ysb,
                        )
                return
            for nch in range(NCH):
                ps = psP.tile([P, 512], fp32, tag="psP", name="ps_o")
                for co in range(KO):
                    mm = nc.tensor.matmul(
                        ps,
                        lhsT=oT_bt[to // TB][:, co, (to % TB) * P : (to % TB + 1) * P],
                        rhs=wo_bf[co][:, nch * 512 : (nch + 1) * 512],
                        start=(co == 0),
                        stop=(co == KO - 1),
                    )
                    dedup_groups[mm.ins.name] = g
                ysb = ypool.tile([P, 512], bf16, tag="y", name="ysb")
                nc.vector.tensor_add(
                    out=ysb, in0=ps, in1=bo_sb[:, nch * 512 : (nch + 1) * 512]
                )
                nc.sync.dma_start(
                    out=y[to * P : (to + 1) * P, nch * 512 : (nch + 1) * 512],
                    in_=ysb,
                )

        # ----- interleaved schedule -----
        # phase 2: per pair m: Q(m), K(m), one V(seq1) pair every other m,
        #          attention (m, b=0)
        n_pairs = H // 2 if attn_lvl > 0 else 0
        for m in range(H // 2):
            if not skip_proj:
                qk_tile(wq_bf, qT_bf, bq_sb, m)
                qk_tile(wk_bf, kT_bf, bk_sb, m)
                if m % 2 == 1:
                    v_tile(TB + m // 2)
            if m < n_pairs:
                attn_pair(m, 0)
        # phase 3: per pair m: attention (m, b=1) + seq-0 out-proj pairs on
        # odd m; `holdback` seq-0 pairs stay in reserve so the PE has ready
        # work while the final pair's softmax normalization drains
        n_oproj_inter = TB - holdback
        for m in range(H // 2):
            if m < n_pairs:
                attn_pair(m, 1)
            if not skip_oproj and m % 2 == 1 and m // 2 < n_oproj_inter:
                oproj_tile(m // 2)
        # phase 4: held-back seq-0 pairs, then seq-1 out-proj
        if skip_oproj:
            ysb0 = ypool.tile([P, 512], bf16, tag="y")
            nc.vector.memset(ysb0[:], 0.0)
            for to in range(NT // P):
                for nch in range(NCH):
                    nc.sync.dma_start(
                        out=y[to * P : (to + 1) * P, nch * 512 : (nch + 1) * 512],
                        in_=ysb0,
                    )
        else:
            for to in range(n_oproj_inter, TB):
                oproj_tile(to)
            for to in range(TB, NT // P):
                oproj_tile(to)

    if dedup_ldw:
        n = _dedup_ldweights(nc, mybir, dedup_groups)
        # 256 pairs exist at emission when pairing; the scheduler's ordering
        # determines how many stay adjacent.
        if pair_proj:
            assert n >= 100, n
        nc._dedup_count = n
    if split_waits:
        _split_big_waits(nc, mybir, limit=SPLIT_LIMIT)
    return nc


def make_in_maps(inputs):
    import ml_dtypes

    bf = ml_dtypes.bfloat16
    x = np.asarray(inputs["x"], dtype=np.float32)
    s = np.float32(1.0 / np.sqrt(D))
    wT = {}
    for k in "qkvo":
        w = np.asarray(inputs[f"W{k}"], dtype=np.float32)
        if k == "q":
            w = w * s
        wT[k] = np.ascontiguousarray(w.T).astype(bf)
    bq = np.asarray(inputs["bq"], dtype=np.float32) * s
    bk = np.asarray(inputs["bk"], dtype=np.float32)
    bv = np.asarray(inputs["bv"], dtype=np.float32)
    bo = np.asarray(inputs["bo"], dtype=np.float32)

    bm = np.asarray(inputs["rel_pos_bias"], dtype=np.float32)[:, :T, :T].copy()
    iu = np.triu_indices(T, 1)
    bm[:, iu[0], iu[1]] = NEG
    # multiplicative form: exp(S+bias) = exp(S) * exp(bias); causal mask
    # becomes an exact multiplicative zero. Transposed to [h, s, t], then
    # packed per head: block j ([s in 128j..128j+128), t >= 128j) at column
    # offset OFF[j] of a [P, 1280] row-contiguous panel (one DMA per head).
    bmT = np.exp(bm.transpose(0, 2, 1)).astype(bf)  # [h, s, t]
    OFF = [0, 512, 896, 1152]
    bm = np.zeros((H // 2, P, 2560), dtype=bf)
    for j in range(TB):
        wj = T - j * P
        for par in range(2):
            bm[:, :, par * 1280 + OFF[j] : par * 1280 + OFF[j] + wj] = bmT[
                par::2, j * P : (j + 1) * P, j * P :
            ]
    bm = np.ascontiguousarray(bm)

    xT_all = x.reshape(N_CORES, NT, C).transpose(0, 2, 1)
    in_maps = []
    for c in range(N_CORES):
        in_maps.append(
            {
                "xT": np.ascontiguousarray(xT_all[c]).astype(bf),
                "wqT": wT["q"],
                "wkT": wT["k"],
                "wvT": wT["v"],
                "woT": wT["o"],
                "bq": bq,
                "bk": bk,
                "bv": bv,
                "bo": bo,
                "biasm": bm,
            }
        )
    return in_maps


def build_jitted(nc, n_cores=N_CORES):
    """Build a persistent jitted shard_map executable for `nc` (the
    multi-core path of bass2jax.run_bass_via_pjrt, kept resident so repeat
    kernel() calls skip retracing)."""
    import jax
    from jax.experimental.shard_map import shard_map
    from jax.sharding import Mesh, NamedSharding, PartitionSpec

    from concourse import mybir
    from concourse.bass2jax import (
        _bass_exec_p,
        install_neuronx_cc_hook,
        partition_id_tensor,
    )

    install_neuronx_cc_hook()
    partition_name = nc.partition_id_tensor.name if nc.partition_id_tensor else None

    in_names, out_names, out_avals, zero_outs = [], [], [], []
    for alloc in nc.m.functions[0].allocations:
        if not isinstance(alloc, mybir.MemoryLocationSet):
            continue
        name = alloc.memorylocations[0].name
        if alloc.kind == "ExternalInput":
            if name != partition_name:
                in_names.append(name)
        elif alloc.kind == "ExternalOutput":
            out_names.append(name)
            shape = tuple(alloc.tensor_shape)
            dtype = mybir.dt.np(alloc.dtype)
            out_avals.append(jax.core.ShapedArray(shape, dtype))
            zero_outs.append(np.zeros(shape, dtype))
    n_params = len(in_names)
    n_outs = len(out_avals)
    all_in_names = list(in_names) + list(out_names)
    if partition_name is not None:
        all_in_names.append(partition_name)
    donate = tuple(range(n_params, n_params + n_outs))

    def _body(*args):
        operands = list(args)
        if partition_name is not None:
            operands.append(partition_id_tensor())
        outs = _bass_exec_p.bind(
            *operands,
            out_avals=tuple(out_avals),
            in_names=tuple(all_in_names),
            out_names=tuple(out_names),
            lowering_input_output_aliases=(),
            sim_require_finite=True,
            sim_require_nnan=True,
            nc=nc,
        )
        return tuple(outs)

    devices = jax.devices()[:n_cores]
    mesh = Mesh(np.asarray(devices), ("core",))
    in_specs = (PartitionSpec("core"),) * (n_params + n_outs)
    out_specs = (PartitionSpec("core"),) * n_outs
    jitted = jax.jit(
        shard_map(_body, mesh=mesh, in_specs=in_specs, out_specs=out_specs,
                  check_rep=False),
        donate_argnums=donate,
        keep_unused=True,
    )
    sharding = NamedSharding(mesh, PartitionSpec("core"))
    return jitted, in_names, out_names, out_avals, zero_outs, sharding


def get_runner():
    """Build the program + executable once; return in_maps -> per-core
    output dicts."""
    if "runner" in _CACHE:
        return _CACHE["runner"]
    import jax

    nc = build_program()
    jitted, in_names, out_names, out_avals, zero_outs, sharding = build_jitted(nc)
    n_cores = N_CORES

    def runner(in_maps):
        concat_in = [
            jax.device_put(
                np.concatenate(
                    [np.asarray(in_maps[c][nm]) for c in range(n_cores)], axis=0
                ),
                sharding,
            )
            for nm in in_names
        ]
        zeros = [
            jax.device_put(
                np.zeros((n_cores * z.shape[0], *z.shape[1:]), z.dtype), sharding
            )
            for z in zero_outs
        ]
        out_arrs = jitted(*concat_in, *zeros)
        return [
            {
                nm: np.asarray(out_arrs[i]).reshape(n_cores, *out_avals[i].shape)[c]
                for i, nm in enumerate(out_names)
            }
            for c in range(n_cores)
        ]

    _CACHE["runner"] = runner
    _CACHE["nc"] = nc
    return runner


def kernel(**inputs) -> np.ndarray:
    runner = get_runner()
    in_maps = make_in_maps(inputs)
    results = runner(in_maps)
    out = np.concatenate(
        [results[c]["y"].reshape(B_LOC, T, C) for c in range(N_CORES)], axis=0
    )
    return out.astype(np.float32)

